# revision 4
# baseline (speedup 1.0000x reference)
"""
LongConvolution (causal FFT conv) Trainium2 Bass kernel.

Problem: x (4, 8192, 1024) f32, filt (1024, 8192) f32.
  y[b, l, c] = sum_m x[b, m, c] * filt[c, l - m]   (causal, per-channel)
Reference computes this via zero-padded FFT of size N = 16384.

Strategy
--------
N = 16384 = 128 * 128 -> four-step FFT where each 128-point DFT stage is a
128x128 matmul on the tensor engine.  With n = 128*n1 + n2, k = 128*k2 + k1:

  A[k1,n2]  = sum_n1 w128^(n1 k1) u[128 n1 + n2]          (matmul vs DFT-128)
  B         = A * T,  T[k1,n2] = wN^(n2 k1)               (twiddle, DVE)
  R[k1,k2]  = sum_n2 B[k1,n2] w128^(n2 k2)                (matmul)
  R^T[k2,k1] = FFT_N(u_pad)[128 k2 + k1]  -> scrambled layout = fft().reshape!
  P = R * K  (filter spectrum K precomputed on HOST in the same layout)
  inverse: mirror image with conj twiddles; only first 64 output rows needed.

Stationary operands alternate between data (F1, I1) and constant DFT matrices
(F2, I2), which makes every stage's input layout exactly what the previous
stage produced - zero on-chip transposes.

Sharding: d_model across the 8 cores (128 channels each); each core handles
all 4 batches of its channels (filter spectrum reused across batch).

Host pre/post: x is transposed per-core to (c, b, l) so every DMA is >=512B
contiguous; output comes back as (c, b, l) and is transposed into (b, l, c).
"""

import os
import sys

import numpy as np

for p in ("/opt/trn_rl_repo",):
    if p not in sys.path:
        sys.path.insert(0, p)

os.environ.setdefault("MYCRO_LOCAL_CACHE", "1")

# ----------------------------------------------------------------------------
# configuration
# ----------------------------------------------------------------------------
B, L, D = 4, 8192, 1024
NFFT = 2 * L               # 16384 = 128 * 128
NC = 8                     # cores
CPC = D // NC              # channels per core = 128

# dtype config: "f32" (exact, slow) or fast variants
MM_DT = os.environ.get("LC_MM_DT", "f32")   # F1 matmul family: f32 | f32r
TT_DT = os.environ.get("LC_TT_DT", "f32")   # elementwise + F2/I1/I2: f32 | f16 | bf16


def _consts():
    """DFT / twiddle constant matrices (float64 -> cast at use site)."""
    j = np.arange(128)
    ang128 = 2 * np.pi * np.outer(j, j) / 128
    angN = 2 * np.pi * np.outer(j, j) / NFFT
    c = {}
    c["F_cos"] = np.cos(ang128)
    c["F_sin"] = np.sin(ang128)
    c["Tw_cos"] = np.cos(angN)
    c["Tw_sin"] = np.sin(angN)
    return c


def _build_program():
    import concourse.bacc as bacc
    import concourse.bass as bass
    import concourse.mybir as mybir
    from concourse import tile

    f32 = mybir.dt.float32
    dt_mm = {"f32": mybir.dt.float32, "f32r": mybir.dt.float32r}[MM_DT]
    dt_tt = {
        "f32": mybir.dt.float32,
        "f16": mybir.dt.float16,
        "bf16": mybir.dt.bfloat16,
    }[TT_DT]
    cast_tt = TT_DT != "f32"

    nc = bacc.Bacc(None, target_bir_lowering=False, debug=False)

    # --- DRAM I/O ---
    xw = nc.dram_tensor("xw", (CPC, B, L), dt_mm, kind="ExternalInput")
    kfre = nc.dram_tensor("kfre", (CPC, 128, 128), dt_tt, kind="ExternalInput")
    kfim = nc.dram_tensor("kfim", (CPC, 128, 128), dt_tt, kind="ExternalInput")
    f1mov_d = nc.dram_tensor("f1mov", (128, 256), dt_mm, kind="ExternalInput")
    f2re_d = nc.dram_tensor("f2re", (128, 128), dt_tt, kind="ExternalInput")
    f2im_d = nc.dram_tensor("f2im", (128, 128), dt_tt, kind="ExternalInput")
    fcmov_d = nc.dram_tensor("fcmov", (128, 384), dt_tt, kind="ExternalInput")
    gre_d = nc.dram_tensor("gre", (128, 64), dt_tt, kind="ExternalInput")
    gimn_d = nc.dram_tensor("gimn", (128, 64), dt_tt, kind="ExternalInput")
    t1re_d = nc.dram_tensor("t1re2", (128, 256), dt_tt, kind="ExternalInput")
    t1im_d = nc.dram_tensor("t1im2", (128, 256), dt_tt, kind="ExternalInput")
    yw = nc.dram_tensor("yw", (CPC, B, L), f32, kind="ExternalOutput")

    with tile.TileContext(nc) as tc:
        with (
            tc.tile_pool(name="const", bufs=1) as constp,
            tc.tile_pool(name="kf", bufs=2) as kfp,
            tc.tile_pool(name="m", bufs=6) as mp,
            tc.tile_pool(name="work", bufs=3) as wp,
            tc.tile_pool(name="out", bufs=3) as op,
            tc.tile_pool(name="pa", bufs=2, space="PSUM") as pap,
            tc.tile_pool(name="pr", bufs=2, space="PSUM") as prp,
            tc.tile_pool(name="pc", bufs=2, space="PSUM") as pcp,
            tc.tile_pool(name="py", bufs=2, space="PSUM") as pyp,
        ):
            # constants, DMA'd once
            f1mov = constp.tile([128, 256], dt_mm)
            f2re = constp.tile([128, 128], dt_tt)
            f2im = constp.tile([128, 128], dt_tt)
            fcmov = constp.tile([128, 384], dt_tt)
            gre = constp.tile([128, 64], dt_tt)
            gimn = constp.tile([128, 64], dt_tt)
            t1re = constp.tile([128, 2, 128], dt_tt)
            t1im = constp.tile([128, 2, 128], dt_tt)
            nc.sync.dma_start(f1mov[:], f1mov_d[:])
            nc.sync.dma_start(f2re[:], f2re_d[:])
            nc.sync.dma_start(f2im[:], f2im_d[:])
            nc.sync.dma_start(fcmov[:], fcmov_d[:])
            nc.sync.dma_start(gre[:], gre_d[:])
            nc.sync.dma_start(gimn[:], gimn_d[:])
            nc.sync.dma_start(
                t1re[:], t1re_d[:].rearrange("p (s n) -> p s n", n=128)
            )
            nc.sync.dma_start(
                t1im[:], t1im_d[:].rearrange("p (s n) -> p s n", n=128)
            )

            for c in range(CPC):
                # filter spectrum for this channel, duplicated along the pair
                # axis so pointwise ops on 2-signal groups see matching shapes
                kre = kfp.tile([128, 2, 128], dt_tt, tag="kre")
                kim = kfp.tile([128, 2, 128], dt_tt, tag="kim")
                for s in range(2):
                    nc.sync.dma_start(kre[:, s, :], kfre[c][:])
                    nc.sync.dma_start(kim[:, s, :], kfim[c][:])

                for p in range(B // 2):  # pairs of batch signals
                    pa = pap.tile([128, 2, 256], f32, tag="pa")
                    # F1: A^T = M^T @ [F_re | F_im-],  K = 64 (upper half zero)
                    for j in range(2):
                        m_t = mp.tile([64, 128], dt_mm, tag="m")
                        nc.sync.dma_start(
                            m_t[:],
                            xw[c, 2 * p + j].rearrange("(a b) -> a b", b=128),
                        )
                        nc.tensor.matmul(
                            pa[:, j, :], m_t[:], f1mov[0:64, :],
                            start=True, stop=True,
                        )

                    # T1 twiddle: B = A * T1  (+ negated imag for F2)
                    if cast_tt:
                        asrc = wp.tile([128, 2, 256], dt_tt, tag="asb")
                        nc.scalar.copy(out=asrc[:], in_=pa[:])
                    else:
                        asrc = pa
                    a_re = asrc[:, :, 0:128]
                    a_im = asrc[:, :, 128:256]
                    u1 = wp.tile([128, 2, 128], dt_tt, tag="u1")
                    u2 = wp.tile([128, 2, 128], dt_tt, tag="u2")
                    u3 = wp.tile([128, 2, 128], dt_tt, tag="u3")
                    u4 = wp.tile([128, 2, 128], dt_tt, tag="u4")
                    b_t = wp.tile([128, 2, 384], dt_tt, tag="b")
                    nc.vector.tensor_mul(u1[:], a_re, t1re[:])
                    nc.vector.tensor_mul(u2[:], a_im, t1im[:])
                    nc.vector.tensor_sub(b_t[:, :, 128:256], u1[:], u2[:])
                    nc.vector.tensor_mul(u3[:], a_re, t1im[:])
                    nc.vector.tensor_mul(u4[:], a_im, t1re[:])
                    nc.vector.tensor_add(b_t[:, :, 256:384], u3[:], u4[:])
                    # B_imn = -B_im (for the [B_imn | B_re] rhs of F2's mm2)
                    nc.vector.tensor_scalar_mul(
                        b_t[:, :, 0:128], b_t[:, :, 256:384], -1.0
                    )

                    # F2: R^T = F- @ B^T   [k2, k1]
                    pr = prp.tile([128, 2, 256], f32, tag="pr")
                    for j in range(2):
                        nc.tensor.matmul(
                            pr[:, j, :], f2re[:], b_t[:, j, 128:384],
                            start=True, stop=False,
                        )
                        nc.tensor.matmul(
                            pr[:, j, :], f2im[:], b_t[:, j, 0:256],
                            start=False, stop=True,
                        )

                    # pointwise with filter spectrum: P = R * K
                    if cast_tt:
                        rsrc = wp.tile([128, 2, 256], dt_tt, tag="rsb")
                        nc.scalar.copy(out=rsrc[:], in_=pr[:])
                    else:
                        rsrc = pr
                    r_re = rsrc[:, :, 0:128]
                    r_im = rsrc[:, :, 128:256]
                    v1 = wp.tile([128, 2, 128], dt_tt, tag="u1")
                    v2 = wp.tile([128, 2, 128], dt_tt, tag="u2")
                    v3 = wp.tile([128, 2, 128], dt_tt, tag="u3")
                    v4 = wp.tile([128, 2, 128], dt_tt, tag="u4")
                    p_re = wp.tile([128, 2, 128], dt_tt, tag="pre")
                    p_im = wp.tile([128, 2, 128], dt_tt, tag="pim")
                    nc.vector.tensor_mul(v1[:], r_re, kre[:])
                    nc.vector.tensor_mul(v2[:], r_im, kim[:])
                    nc.vector.tensor_sub(p_re[:], v1[:], v2[:])
                    nc.vector.tensor_mul(v3[:], r_re, kim[:])
                    nc.vector.tensor_mul(v4[:], r_im, kre[:])
                    nc.vector.tensor_add(p_im[:], v3[:], v4[:])

                    # I1: C = P @ F+   [k1, n2]  (data-stationary)
                    pc = pcp.tile([128, 2, 256], f32, tag="pc")
                    for j in range(2):
                        nc.tensor.matmul(
                            pc[:, j, :], p_re[:, j, :], fcmov[:, 128:384],
                            start=True, stop=False,
                        )
                        nc.tensor.matmul(
                            pc[:, j, :], p_im[:, j, :], fcmov[:, 0:256],
                            start=False, stop=True,
                        )

                    # T2 twiddle: C' = C * conj(T1)
                    if cast_tt:
                        csrc = wp.tile([128, 2, 256], dt_tt, tag="csb")
                        nc.scalar.copy(out=csrc[:], in_=pc[:])
                    else:
                        csrc = pc
                    c_re = csrc[:, :, 0:128]
                    c_im = csrc[:, :, 128:256]
                    w1 = wp.tile([128, 2, 128], dt_tt, tag="u1")
                    w2 = wp.tile([128, 2, 128], dt_tt, tag="u2")
                    w3 = wp.tile([128, 2, 128], dt_tt, tag="u3")
                    w4 = wp.tile([128, 2, 128], dt_tt, tag="u4")
                    cp_re = wp.tile([128, 2, 128], dt_tt, tag="cpre")
                    cp_im = wp.tile([128, 2, 128], dt_tt, tag="cpim")
                    nc.vector.tensor_mul(w1[:], c_re, t1re[:])
                    nc.vector.tensor_mul(w2[:], c_im, t1im[:])
                    nc.vector.tensor_add(cp_re[:], w1[:], w2[:])
                    nc.vector.tensor_mul(w3[:], c_re, t1im[:])
                    nc.vector.tensor_mul(w4[:], c_im, t1re[:])
                    nc.vector.tensor_sub(cp_im[:], w4[:], w3[:])

                    # I2: y = Re(F+ @ C') / N, first 64 rows only
                    py = pyp.tile([64, 2, 128], f32, tag="py")
                    nc.tensor.matmul(
                        py[:], gre[:], cp_re[:], start=True, stop=False
                    )
                    nc.tensor.matmul(
                        py[:], gimn[:], cp_im[:], start=False, stop=True
                    )

                    ysb = op.tile([64, 2, 128], f32, tag="ysb")
                    nc.scalar.copy(out=ysb[:], in_=py[:])
                    for j in range(2):
                        nc.sync.dma_start(
                            yw[c, 2 * p + j].rearrange("(a b) -> a b", b=128),
                            ysb[:, j, :],
                        )

    nc.compile()
    return nc


def _host_arrays():
    cst = _consts()
    F_cos, F_sin = cst["F_cos"], cst["F_sin"]
    Tw_cos, Tw_sin = cst["Tw_cos"], cst["Tw_sin"]

    np_tt = {"f32": np.float32, "f16": np.float16, "bf16": None}[TT_DT]
    if np_tt is None:
        import ml_dtypes

        np_tt = ml_dtypes.bfloat16
    np_mm = np.float32

    arrs = {}
    arrs["f1mov"] = np.concatenate([F_cos, -F_sin], axis=1).astype(np_mm)
    arrs["f2re"] = F_cos.astype(np_tt)
    arrs["f2im"] = (-F_sin).astype(np_tt)
    # fcmov = [F+_im_neg | F+_re | F+_im] = [-sin | cos | sin]
    arrs["fcmov"] = np.concatenate([-F_sin, F_cos, F_sin], axis=1).astype(np_tt)
    # 1/NFFT normalization lives in the host-side filter spectrum (keeps
    # every on-chip intermediate within fp16 range)
    arrs["gre"] = F_cos[:, :64].astype(np_tt)
    arrs["gimn"] = (-F_sin[:, :64]).astype(np_tt)
    arrs["t1re2"] = np.concatenate([Tw_cos, Tw_cos], axis=1).astype(np_tt)
    arrs["t1im2"] = np.concatenate([-Tw_sin, -Tw_sin], axis=1).astype(np_tt)
    return arrs, np_tt


def kernel(x: np.ndarray, filt: np.ndarray) -> np.ndarray:
    from concourse.bass_utils import run_bass_kernel_spmd

    assert x.shape == (B, L, D) and filt.shape == (D, L)
    x = np.ascontiguousarray(x, dtype=np.float32)
    filt = np.ascontiguousarray(filt, dtype=np.float32)

    consts, np_tt = _host_arrays()

    # filter spectrum: FFT of zero-padded filter; reshape(128,128) IS the
    # scrambled [k2,k1] layout produced by the on-device four-step forward.
    kpad = np.zeros((D, NFFT), np.float64)
    kpad[:, :L] = filt
    Kf = (np.fft.fft(kpad, axis=1) / NFFT).reshape(D, 128, 128)

    in_maps = []
    for ci in range(NC):
        sl = slice(ci * CPC, (ci + 1) * CPC)
        m = dict(consts)
        m["xw"] = np.ascontiguousarray(x[:, :, sl].transpose(2, 0, 1))
        m["kfre"] = np.ascontiguousarray(Kf[sl].real.astype(np_tt))
        m["kfim"] = np.ascontiguousarray(Kf[sl].imag.astype(np_tt))
        in_maps.append(m)

    nc = _build_program()
    res = run_bass_kernel_spmd(nc, in_maps, core_ids=list(range(NC)))

    y = np.empty((B, L, D), np.float32)
    for ci in range(NC):
        sl = slice(ci * CPC, (ci + 1) * CPC)
        y[:, :, sl] = res.results[ci]["yw"].transpose(1, 2, 0)
    return y


def run_profiled(inputs):
    """Build + run with NTFF tracing; returns BassKernelResults (test-only)."""
    from concourse.bass_utils import run_bass_kernel_spmd

    x = np.ascontiguousarray(inputs["x"], dtype=np.float32)
    filt = np.ascontiguousarray(inputs["filt"], dtype=np.float32)
    consts, np_tt = _host_arrays()
    kpad = np.zeros((D, NFFT), np.float64)
    kpad[:, :L] = filt
    Kf = (np.fft.fft(kpad, axis=1) / NFFT).reshape(D, 128, 128)
    in_maps = []
    for ci in range(NC):
        sl = slice(ci * CPC, (ci + 1) * CPC)
        m = dict(consts)
        m["xw"] = np.ascontiguousarray(x[:, :, sl].transpose(2, 0, 1))
        m["kfre"] = np.ascontiguousarray(Kf[sl].real.astype(np_tt))
        m["kfim"] = np.ascontiguousarray(Kf[sl].imag.astype(np_tt))
        in_maps.append(m)
    nc = _build_program()
    return run_bass_kernel_spmd(
        nc, in_maps, core_ids=list(range(NC)), trace=True
    )


if __name__ == "__main__":
    rng = np.random.default_rng(0)
    x = rng.standard_normal((B, L, D)).astype(np.float32)
    filt = rng.standard_normal((D, L)).astype(np.float32)
    y = kernel(x, filt)
    print("y", y.shape, y.dtype, float(np.abs(y).max()))


# revision 7
# speedup vs baseline: 1.2551x; 1.2551x over previous
"""
LongConvolution (causal FFT conv) Trainium2 Bass kernel.

Problem: x (4, 8192, 1024) f32, filt (1024, 8192) f32.
  y[b, l, c] = sum_m x[b, m, c] * filt[c, l - m]   (causal, per-channel)
Reference computes this via zero-padded FFT of size N = 16384.

Strategy
--------
N = 16384 = 128 * 128 -> four-step FFT where each 128-point DFT stage is a
128x128 matmul on the tensor engine.  With n = 128*n1 + n2, k = 128*k2 + k1:

  A[k1,n2]  = sum_n1 w128^(n1 k1) u[128 n1 + n2]          (matmul vs DFT-128)
  B         = A * T,  T[k1,n2] = wN^(n2 k1)               (twiddle, DVE)
  R[k1,k2]  = sum_n2 B[k1,n2] w128^(n2 k2)                (matmul)
  R^T[k2,k1] = FFT_N(u_pad)[128 k2 + k1]  -> scrambled layout = fft().reshape!
  P = R * K  (filter spectrum K precomputed on HOST in the same layout)
  inverse: mirror image with conj twiddles; only first 64 output rows needed.

Stationary operands alternate between data (F1, I1) and constant DFT matrices
(F2, I2), which makes every stage's input layout exactly what the previous
stage produced - zero on-chip transposes.

Sharding: d_model across the 8 cores (128 channels each); each core handles
all 4 batches of its channels (filter spectrum reused across batch).

Host pre/post: x is transposed per-core to (c, b, l) so every DMA is >=512B
contiguous; output comes back as (c, b, l) and is transposed into (b, l, c).
"""

import os
import sys

import numpy as np

for p in ("/opt/trn_rl_repo",):
    if p not in sys.path:
        sys.path.insert(0, p)

os.environ.setdefault("MYCRO_LOCAL_CACHE", "1")

# ----------------------------------------------------------------------------
# configuration
# ----------------------------------------------------------------------------
B, L, D = 4, 8192, 1024
NFFT = 2 * L               # 16384 = 128 * 128
NC = 8                     # cores
CPC = D // NC              # channels per core = 128

# dtype config: "f32" (exact, slow) or fast variants
MM_DT = os.environ.get("LC_MM_DT", "f32")   # F1 matmul family: f32 | f32r
TT_DT = os.environ.get("LC_TT_DT", "f32")   # elementwise + F2/I1/I2: f32 | f16 | bf16


def _consts():
    """DFT / twiddle constant matrices (float64 -> cast at use site)."""
    j = np.arange(128)
    ang128 = 2 * np.pi * np.outer(j, j) / 128
    angN = 2 * np.pi * np.outer(j, j) / NFFT
    c = {}
    c["F_cos"] = np.cos(ang128)
    c["F_sin"] = np.sin(ang128)
    c["Tw_cos"] = np.cos(angN)
    c["Tw_sin"] = np.sin(angN)
    return c


def _build_program():
    import concourse.bacc as bacc
    import concourse.bass as bass
    import concourse.mybir as mybir
    from concourse import tile

    f32 = mybir.dt.float32
    dt_mm = {"f32": mybir.dt.float32, "f32r": mybir.dt.float32r}[MM_DT]
    dt_tt = {
        "f32": mybir.dt.float32,
        "f16": mybir.dt.float16,
        "bf16": mybir.dt.bfloat16,
    }[TT_DT]
    cast_tt = TT_DT != "f32"

    nc = bacc.Bacc(None, target_bir_lowering=False, debug=False)

    # --- DRAM I/O ---
    xw = nc.dram_tensor("xw", (CPC, B, L), dt_mm, kind="ExternalInput")
    kfre = nc.dram_tensor("kfre", (CPC, 128, 128), dt_tt, kind="ExternalInput")
    kfim = nc.dram_tensor("kfim", (CPC, 128, 128), dt_tt, kind="ExternalInput")
    f1mov_d = nc.dram_tensor("f1mov", (128, 256), dt_mm, kind="ExternalInput")
    f2re_d = nc.dram_tensor("f2re", (128, 128), dt_tt, kind="ExternalInput")
    f2im_d = nc.dram_tensor("f2im", (128, 128), dt_tt, kind="ExternalInput")
    f2sin_d = nc.dram_tensor("f2sin", (128, 128), dt_tt, kind="ExternalInput")
    fcmov_d = nc.dram_tensor("fcmov", (128, 384), dt_tt, kind="ExternalInput")
    gre_d = nc.dram_tensor("gre", (128, 64), dt_tt, kind="ExternalInput")
    gimn_d = nc.dram_tensor("gimn", (128, 64), dt_tt, kind="ExternalInput")
    t1re_d = nc.dram_tensor("t1re2", (128, 128), dt_tt, kind="ExternalInput")
    t1im_d = nc.dram_tensor("t1im2", (128, 128), dt_tt, kind="ExternalInput")
    yw = nc.dram_tensor("yw", (CPC, B, L), f32, kind="ExternalOutput")

    G = B  # all 4 batch signals of a channel processed as one group

    with tile.TileContext(nc) as tc:
        with (
            tc.tile_pool(name="const", bufs=1) as constp,
            tc.tile_pool(name="kf", bufs=3) as kfp,
            tc.tile_pool(name="m", bufs=3) as mp,
            tc.tile_pool(name="work", bufs=3) as wp,
            tc.tile_pool(name="out", bufs=3) as op,
            tc.tile_pool(name="pa", bufs=1, space="PSUM") as pap,
            tc.tile_pool(name="pr", bufs=1, space="PSUM") as prp,
            tc.tile_pool(name="pc", bufs=1, space="PSUM") as pcp,
            tc.tile_pool(name="py", bufs=1, space="PSUM") as pyp,
        ):
            # constants, DMA'd once
            f1mov = constp.tile([128, 256], dt_mm)
            f2re = constp.tile([128, 128], dt_tt)
            f2im = constp.tile([128, 128], dt_tt)
            f2sin = constp.tile([128, 128], dt_tt)
            fcmov = constp.tile([128, 384], dt_tt)
            gre = constp.tile([128, 64], dt_tt)
            gimn = constp.tile([128, 64], dt_tt)
            t1re = constp.tile([128, 128], dt_tt)
            t1im = constp.tile([128, 128], dt_tt)
            nc.sync.dma_start(f1mov[:], f1mov_d[:])
            nc.sync.dma_start(f2re[:], f2re_d[:])
            nc.sync.dma_start(f2im[:], f2im_d[:])
            nc.sync.dma_start(f2sin[:], f2sin_d[:])
            nc.sync.dma_start(fcmov[:], fcmov_d[:])
            nc.sync.dma_start(gre[:], gre_d[:])
            nc.sync.dma_start(gimn[:], gimn_d[:])
            nc.sync.dma_start(t1re[:], t1re_d[:])
            nc.sync.dma_start(t1im[:], t1im_d[:])
            t1re_b = t1re[:].rearrange("p (s n) -> p s n", s=1).broadcast_to([128, G, 128])
            t1im_b = t1im[:].rearrange("p (s n) -> p s n", s=1).broadcast_to([128, G, 128])

            for c in range(CPC):
                kre = kfp.tile([128, 128], dt_tt, tag="kre")
                kim = kfp.tile([128, 128], dt_tt, tag="kim")
                nc.sync.dma_start(kre[:], kfre[c][:])
                nc.sync.dma_start(kim[:], kfim[c][:])
                kre_b = kre[:].rearrange("p (s n) -> p s n", s=1).broadcast_to([128, G, 128])
                kim_b = kim[:].rearrange("p (s n) -> p s n", s=1).broadcast_to([128, G, 128])

                # F1: A^T = M^T @ [F_re | F_im-],  K = 64 (upper half zero)
                m4 = mp.tile([64, G, 128], dt_mm, tag="m")
                nc.sync.dma_start(
                    m4[:], xw[c].rearrange("b (a n) -> a b n", n=128)
                )
                pa = pap.tile([128, G, 256], f32, tag="pa")
                for j in range(G):
                    nc.tensor.matmul(
                        pa[:, j, :], m4[:, j, :], f1mov[0:64, :],
                        start=True, stop=True,
                    )

                # T1 twiddle: B = A * T1
                if cast_tt:
                    asrc = wp.tile([128, G, 256], dt_tt, tag="asb")
                    nc.scalar.copy(out=asrc[:], in_=pa[:])
                else:
                    asrc = pa
                a_re = asrc[:, :, 0:128]
                a_im = asrc[:, :, 128:256]
                u1 = wp.tile([128, G, 128], dt_tt, tag="u1")
                u2 = wp.tile([128, G, 128], dt_tt, tag="u2")
                u3 = wp.tile([128, G, 128], dt_tt, tag="u3")
                u4 = wp.tile([128, G, 128], dt_tt, tag="u4")
                b_t = wp.tile([128, G, 256], dt_tt, tag="b")
                nc.vector.tensor_mul(u1[:], a_re, t1re_b)
                nc.vector.tensor_mul(u2[:], a_im, t1im_b)
                nc.vector.tensor_sub(b_t[:, :, 0:128], u1[:], u2[:])
                nc.vector.tensor_mul(u3[:], a_re, t1im_b)
                nc.vector.tensor_mul(u4[:], a_im, t1re_b)
                nc.vector.tensor_add(b_t[:, :, 128:256], u3[:], u4[:])

                # F2: R^T = F- @ B^T  [k2, k1]; sign of the sin-part lives in
                # the constants (f2sin / f2im), so no negated-B tile is needed
                pr = prp.tile([128, G, 256], f32, tag="pr")
                for g in range(G // 2):  # one psum bank per 2 signals
                    sl = slice(2 * g, 2 * g + 2)
                    b_re = b_t[:, sl, 0:128]
                    b_im = b_t[:, sl, 128:256]
                    nc.tensor.matmul(
                        pr[:, sl, 0:128], f2re[:], b_re, start=True, stop=False
                    )
                    nc.tensor.matmul(
                        pr[:, sl, 0:128], f2sin[:], b_im, start=False, stop=True
                    )
                    nc.tensor.matmul(
                        pr[:, sl, 128:256], f2re[:], b_im, start=True, stop=False
                    )
                    nc.tensor.matmul(
                        pr[:, sl, 128:256], f2im[:], b_re, start=False, stop=True
                    )

                # pointwise with filter spectrum: P = R * K
                if cast_tt:
                    rsrc = wp.tile([128, G, 256], dt_tt, tag="rsb")
                    nc.scalar.copy(out=rsrc[:], in_=pr[:])
                else:
                    rsrc = pr
                r_re = rsrc[:, :, 0:128]
                r_im = rsrc[:, :, 128:256]
                v1 = wp.tile([128, G, 128], dt_tt, tag="u1")
                v2 = wp.tile([128, G, 128], dt_tt, tag="u2")
                v3 = wp.tile([128, G, 128], dt_tt, tag="u3")
                v4 = wp.tile([128, G, 128], dt_tt, tag="u4")
                p_re = wp.tile([128, G, 128], dt_tt, tag="pre")
                p_im = wp.tile([128, G, 128], dt_tt, tag="pim")
                nc.vector.tensor_mul(v1[:], r_re, kre_b)
                nc.vector.tensor_mul(v2[:], r_im, kim_b)
                nc.vector.tensor_sub(p_re[:], v1[:], v2[:])
                nc.vector.tensor_mul(v3[:], r_re, kim_b)
                nc.vector.tensor_mul(v4[:], r_im, kre_b)
                nc.vector.tensor_add(p_im[:], v3[:], v4[:])

                # I1: C = P @ F+   [k1, n2]  (data-stationary)
                pc = pcp.tile([128, G, 256], f32, tag="pc")
                for j in range(G):
                    nc.tensor.matmul(
                        pc[:, j, :], p_re[:, j, :], fcmov[:, 128:384],
                        start=True, stop=False,
                    )
                    nc.tensor.matmul(
                        pc[:, j, :], p_im[:, j, :], fcmov[:, 0:256],
                        start=False, stop=True,
                    )

                # T2 twiddle: C' = C * conj(T1)
                if cast_tt:
                    csrc = wp.tile([128, G, 256], dt_tt, tag="csb")
                    nc.scalar.copy(out=csrc[:], in_=pc[:])
                else:
                    csrc = pc
                c_re = csrc[:, :, 0:128]
                c_im = csrc[:, :, 128:256]
                w1 = wp.tile([128, G, 128], dt_tt, tag="u1")
                w2 = wp.tile([128, G, 128], dt_tt, tag="u2")
                w3 = wp.tile([128, G, 128], dt_tt, tag="u3")
                w4 = wp.tile([128, G, 128], dt_tt, tag="u4")
                cp_re = wp.tile([128, G, 128], dt_tt, tag="cpre")
                cp_im = wp.tile([128, G, 128], dt_tt, tag="cpim")
                nc.vector.tensor_mul(w1[:], c_re, t1re_b)
                nc.vector.tensor_mul(w2[:], c_im, t1im_b)
                nc.vector.tensor_add(cp_re[:], w1[:], w2[:])
                nc.vector.tensor_mul(w3[:], c_re, t1im_b)
                nc.vector.tensor_mul(w4[:], c_im, t1re_b)
                nc.vector.tensor_sub(cp_im[:], w4[:], w3[:])

                # I2: y = Re(F+ @ C'), first 64 rows; 1/N folded into K
                py = pyp.tile([64, G, 128], f32, tag="py")
                nc.tensor.matmul(py[:], gre[:], cp_re[:], start=True, stop=False)
                nc.tensor.matmul(py[:], gimn[:], cp_im[:], start=False, stop=True)

                ysb = op.tile([64, G, 128], f32, tag="ysb")
                nc.scalar.copy(out=ysb[:], in_=py[:])
                nc.sync.dma_start(
                    yw[c].rearrange("b (a n) -> a b n", n=128), ysb[:]
                )

    nc.compile()
    return nc


def _host_arrays():
    cst = _consts()
    F_cos, F_sin = cst["F_cos"], cst["F_sin"]
    Tw_cos, Tw_sin = cst["Tw_cos"], cst["Tw_sin"]

    np_tt = {"f32": np.float32, "f16": np.float16, "bf16": None}[TT_DT]
    if np_tt is None:
        import ml_dtypes

        np_tt = ml_dtypes.bfloat16
    np_mm = np.float32

    arrs = {}
    arrs["f1mov"] = np.concatenate([F_cos, -F_sin], axis=1).astype(np_mm)
    arrs["f2re"] = F_cos.astype(np_tt)
    arrs["f2im"] = (-F_sin).astype(np_tt)
    arrs["f2sin"] = F_sin.astype(np_tt)
    # fcmov = [F+_im_neg | F+_re | F+_im] = [-sin | cos | sin]
    arrs["fcmov"] = np.concatenate([-F_sin, F_cos, F_sin], axis=1).astype(np_tt)
    # 1/NFFT normalization lives in the host-side filter spectrum (keeps
    # every on-chip intermediate within fp16 range)
    arrs["gre"] = F_cos[:, :64].astype(np_tt)
    arrs["gimn"] = (-F_sin[:, :64]).astype(np_tt)
    arrs["t1re2"] = Tw_cos.astype(np_tt)
    arrs["t1im2"] = (-Tw_sin).astype(np_tt)
    return arrs, np_tt


def kernel(x: np.ndarray, filt: np.ndarray) -> np.ndarray:
    from concourse.bass_utils import run_bass_kernel_spmd

    assert x.shape == (B, L, D) and filt.shape == (D, L)
    x = np.ascontiguousarray(x, dtype=np.float32)
    filt = np.ascontiguousarray(filt, dtype=np.float32)

    consts, np_tt = _host_arrays()

    # filter spectrum: FFT of zero-padded filter; reshape(128,128) IS the
    # scrambled [k2,k1] layout produced by the on-device four-step forward.
    kpad = np.zeros((D, NFFT), np.float64)
    kpad[:, :L] = filt
    Kf = (np.fft.fft(kpad, axis=1) / NFFT).reshape(D, 128, 128)

    in_maps = []
    for ci in range(NC):
        sl = slice(ci * CPC, (ci + 1) * CPC)
        m = dict(consts)
        m["xw"] = np.ascontiguousarray(x[:, :, sl].transpose(2, 0, 1))
        m["kfre"] = np.ascontiguousarray(Kf[sl].real.astype(np_tt))
        m["kfim"] = np.ascontiguousarray(Kf[sl].imag.astype(np_tt))
        in_maps.append(m)

    nc = _build_program()
    res = run_bass_kernel_spmd(nc, in_maps, core_ids=list(range(NC)))

    y = np.empty((B, L, D), np.float32)
    for ci in range(NC):
        sl = slice(ci * CPC, (ci + 1) * CPC)
        y[:, :, sl] = res.results[ci]["yw"].transpose(1, 2, 0)
    return y


def run_profiled(inputs):
    """Build + run with NTFF tracing; returns BassKernelResults (test-only)."""
    from concourse.bass_utils import run_bass_kernel_spmd

    x = np.ascontiguousarray(inputs["x"], dtype=np.float32)
    filt = np.ascontiguousarray(inputs["filt"], dtype=np.float32)
    consts, np_tt = _host_arrays()
    kpad = np.zeros((D, NFFT), np.float64)
    kpad[:, :L] = filt
    Kf = (np.fft.fft(kpad, axis=1) / NFFT).reshape(D, 128, 128)
    in_maps = []
    for ci in range(NC):
        sl = slice(ci * CPC, (ci + 1) * CPC)
        m = dict(consts)
        m["xw"] = np.ascontiguousarray(x[:, :, sl].transpose(2, 0, 1))
        m["kfre"] = np.ascontiguousarray(Kf[sl].real.astype(np_tt))
        m["kfim"] = np.ascontiguousarray(Kf[sl].imag.astype(np_tt))
        in_maps.append(m)
    nc = _build_program()
    return run_bass_kernel_spmd(
        nc, in_maps, core_ids=list(range(NC)), trace=True
    )


if __name__ == "__main__":
    rng = np.random.default_rng(0)
    x = rng.standard_normal((B, L, D)).astype(np.float32)
    filt = rng.standard_normal((D, L)).astype(np.float32)
    y = kernel(x, filt)
    print("y", y.shape, y.dtype, float(np.abs(y).max()))


# revision 9
# speedup vs baseline: 1.4056x; 1.1199x over previous
"""
LongConvolution (causal FFT conv) Trainium2 Bass kernel.

Problem: x (4, 8192, 1024) f32, filt (1024, 8192) f32.
  y[b, l, c] = sum_m x[b, m, c] * filt[c, l - m]   (causal, per-channel)
Reference computes this via zero-padded FFT of size N = 16384.

Strategy
--------
N = 16384 = 128 * 128 -> four-step FFT where each 128-point DFT stage is a
128x128 matmul on the tensor engine.  With n = 128*n1 + n2, k = 128*k2 + k1:

  A[k1,n2]  = sum_n1 w128^(n1 k1) u[128 n1 + n2]          (matmul vs DFT-128)
  B         = A * T,  T[k1,n2] = wN^(n2 k1)               (twiddle, DVE)
  R[k1,k2]  = sum_n2 B[k1,n2] w128^(n2 k2)                (matmul)
  R^T[k2,k1] = FFT_N(u_pad)[128 k2 + k1]  -> scrambled layout = fft().reshape!
  P = R * K  (filter spectrum K precomputed on HOST in the same layout)
  inverse: mirror image with conj twiddles; only first 64 output rows needed.

Stationary operands alternate between data (F1, I1) and constant DFT matrices
(F2, I2), which makes every stage's input layout exactly what the previous
stage produced - zero on-chip transposes.

Sharding: d_model across the 8 cores (128 channels each); each core handles
all 4 batches of its channels (filter spectrum reused across batch).

Host pre/post: x is transposed per-core to (c, b, l) so every DMA is >=512B
contiguous; output comes back as (c, b, l) and is transposed into (b, l, c).
"""

import os
import sys

import numpy as np

for p in ("/opt/trn_rl_repo",):
    if p not in sys.path:
        sys.path.insert(0, p)

os.environ.setdefault("MYCRO_LOCAL_CACHE", "1")

# ----------------------------------------------------------------------------
# configuration
# ----------------------------------------------------------------------------
B, L, D = 4, 8192, 1024
NFFT = 2 * L               # 16384 = 128 * 128
NC = 8                     # cores
CPC = D // NC              # channels per core = 128

# dtype config: "f32" (exact, slow) or fast variants
MM_DT = os.environ.get("LC_MM_DT", "f32")   # F1 matmul family: f32 | f32r
TT_DT = os.environ.get("LC_TT_DT", "f32")   # elementwise + F2/I1/I2: f32 | f16 | bf16


def _consts():
    """DFT / twiddle constant matrices (float64 -> cast at use site)."""
    j = np.arange(128)
    ang128 = 2 * np.pi * np.outer(j, j) / 128
    angN = 2 * np.pi * np.outer(j, j) / NFFT
    c = {}
    c["F_cos"] = np.cos(ang128)
    c["F_sin"] = np.sin(ang128)
    c["Tw_cos"] = np.cos(angN)
    c["Tw_sin"] = np.sin(angN)
    return c


def _build_program():
    import concourse.bacc as bacc
    import concourse.bass as bass
    import concourse.mybir as mybir
    from concourse import tile

    f32 = mybir.dt.float32
    dt_mm = {"f32": mybir.dt.float32, "f32r": mybir.dt.float32r}[MM_DT]
    dt_tt = {
        "f32": mybir.dt.float32,
        "f16": mybir.dt.float16,
        "bf16": mybir.dt.bfloat16,
    }[TT_DT]
    cast_tt = TT_DT != "f32"

    nc = bacc.Bacc(None, target_bir_lowering=False, debug=False)

    # --- DRAM I/O ---
    xw = nc.dram_tensor("xw", (CPC, B, L), dt_mm, kind="ExternalInput")
    kfre = nc.dram_tensor("kfre", (CPC, 128, 128), dt_tt, kind="ExternalInput")
    kfim = nc.dram_tensor("kfim", (CPC, 128, 128), dt_tt, kind="ExternalInput")
    f1mov_d = nc.dram_tensor("f1mov", (128, 256), dt_mm, kind="ExternalInput")
    f2re_d = nc.dram_tensor("f2re", (128, 128), dt_tt, kind="ExternalInput")
    f2im_d = nc.dram_tensor("f2im", (128, 128), dt_tt, kind="ExternalInput")
    f2sin_d = nc.dram_tensor("f2sin", (128, 128), dt_tt, kind="ExternalInput")
    fcmov_d = nc.dram_tensor("fcmov", (128, 384), dt_tt, kind="ExternalInput")
    gre_d = nc.dram_tensor("gre", (128, 64), dt_tt, kind="ExternalInput")
    gimn_d = nc.dram_tensor("gimn", (128, 64), dt_tt, kind="ExternalInput")
    t1re_d = nc.dram_tensor("t1re2", (128, 128), dt_tt, kind="ExternalInput")
    t1im_d = nc.dram_tensor("t1im2", (128, 128), dt_tt, kind="ExternalInput")
    yw = nc.dram_tensor("yw", (CPC, B, L), f32, kind="ExternalOutput")

    G = B  # all 4 batch signals of a channel processed as one group

    with tile.TileContext(nc) as tc:
        with (
            tc.tile_pool(name="const", bufs=1) as constp,
            tc.tile_pool(name="kf", bufs=4) as kfp,
            tc.tile_pool(name="m", bufs=4) as mp,
            tc.tile_pool(name="work", bufs=4) as wp,
            tc.tile_pool(name="out", bufs=4) as op,
            tc.tile_pool(name="pa", bufs=2, space="PSUM") as pap,
            tc.tile_pool(name="pr", bufs=2, space="PSUM") as prp,
            tc.tile_pool(name="pc", bufs=2, space="PSUM") as pcp,
            tc.tile_pool(name="py", bufs=2, space="PSUM") as pyp,
        ):
            # constants, DMA'd once
            f1mov = constp.tile([128, 256], dt_mm)
            f2re = constp.tile([128, 128], dt_tt)
            f2im = constp.tile([128, 128], dt_tt)
            f2sin = constp.tile([128, 128], dt_tt)
            fcmov = constp.tile([128, 384], dt_tt)
            gre = constp.tile([128, 64], dt_tt)
            gimn = constp.tile([128, 64], dt_tt)
            t1re = constp.tile([128, 128], dt_tt)
            t1im = constp.tile([128, 128], dt_tt)
            nc.sync.dma_start(f1mov[:], f1mov_d[:])
            nc.sync.dma_start(f2re[:], f2re_d[:])
            nc.sync.dma_start(f2im[:], f2im_d[:])
            nc.sync.dma_start(f2sin[:], f2sin_d[:])
            nc.sync.dma_start(fcmov[:], fcmov_d[:])
            nc.sync.dma_start(gre[:], gre_d[:])
            nc.sync.dma_start(gimn[:], gimn_d[:])
            nc.sync.dma_start(t1re[:], t1re_d[:])
            nc.sync.dma_start(t1im[:], t1im_d[:])
            t1re_b = t1re[:].rearrange("p (s n) -> p s n", s=1).broadcast_to([128, G, 128])
            t1im_b = t1im[:].rearrange("p (s n) -> p s n", s=1).broadcast_to([128, G, 128])

            for c in range(CPC):
                kre = kfp.tile([128, 128], dt_tt, tag="kre")
                kim = kfp.tile([128, 128], dt_tt, tag="kim")
                nc.sync.dma_start(kre[:], kfre[c][:])
                nc.sync.dma_start(kim[:], kfim[c][:])
                kre_b = kre[:].rearrange("p (s n) -> p s n", s=1).broadcast_to([128, G, 128])
                kim_b = kim[:].rearrange("p (s n) -> p s n", s=1).broadcast_to([128, G, 128])

                # F1: A^T = M^T @ [F_re | F_im-],  K = 64 (upper half zero)
                m4 = mp.tile([64, G, 128], dt_mm, tag="m")
                nc.sync.dma_start(
                    m4[:], xw[c].rearrange("b (a n) -> a b n", n=128)
                )
                asrc = wp.tile([128, G, 256], dt_tt, tag="asb")
                for g in range(G // 2):
                    pa = pap.tile([128, 2, 256], f32, tag="pa")
                    for i in range(2):
                        j = 2 * g + i
                        nc.tensor.matmul(
                            pa[:, i, :], m4[:, j, :], f1mov[0:64, :],
                            start=True, stop=True,
                        )
                    nc.scalar.copy(
                        out=asrc[:, 2 * g : 2 * g + 2, :], in_=pa[:]
                    )

                # T1 twiddle: B = A * T1
                a_re = asrc[:, :, 0:128]
                a_im = asrc[:, :, 128:256]
                u1 = wp.tile([128, G, 128], dt_tt, tag="u1")
                u2 = wp.tile([128, G, 128], dt_tt, tag="u2")
                u3 = wp.tile([128, G, 128], dt_tt, tag="u3")
                u4 = wp.tile([128, G, 128], dt_tt, tag="u4")
                b_t = wp.tile([128, G, 256], dt_tt, tag="b")
                nc.vector.tensor_mul(u1[:], a_re, t1re_b)
                nc.vector.tensor_mul(u2[:], a_im, t1im_b)
                nc.vector.tensor_sub(b_t[:, :, 0:128], u1[:], u2[:])
                nc.vector.tensor_mul(u3[:], a_re, t1im_b)
                nc.vector.tensor_mul(u4[:], a_im, t1re_b)
                nc.vector.tensor_add(b_t[:, :, 128:256], u3[:], u4[:])

                # F2: R^T = F- @ B^T  [k2, k1]; sign of the sin-part lives in
                # the constants (f2sin / f2im), so no negated-B tile is needed
                rsrc = wp.tile([128, G, 256], dt_tt, tag="rsb")
                for g in range(G // 2):  # one psum bank per 2 signals
                    sl = slice(2 * g, 2 * g + 2)
                    b_re = b_t[:, sl, 0:128]
                    b_im = b_t[:, sl, 128:256]
                    pr = prp.tile([128, 2, 256], f32, tag="pr")
                    nc.tensor.matmul(
                        pr[:, :, 0:128], f2re[:], b_re, start=True, stop=False
                    )
                    nc.tensor.matmul(
                        pr[:, :, 0:128], f2sin[:], b_im, start=False, stop=True
                    )
                    nc.tensor.matmul(
                        pr[:, :, 128:256], f2re[:], b_im, start=True, stop=False
                    )
                    nc.tensor.matmul(
                        pr[:, :, 128:256], f2im[:], b_re, start=False, stop=True
                    )
                    nc.scalar.copy(out=rsrc[:, sl, :], in_=pr[:])

                # pointwise with filter spectrum: P = R * K
                r_re = rsrc[:, :, 0:128]
                r_im = rsrc[:, :, 128:256]
                v1 = wp.tile([128, G, 128], dt_tt, tag="u1")
                v2 = wp.tile([128, G, 128], dt_tt, tag="u2")
                v3 = wp.tile([128, G, 128], dt_tt, tag="u3")
                v4 = wp.tile([128, G, 128], dt_tt, tag="u4")
                p_re = wp.tile([128, G, 128], dt_tt, tag="pre")
                p_im = wp.tile([128, G, 128], dt_tt, tag="pim")
                nc.vector.tensor_mul(v1[:], r_re, kre_b)
                nc.vector.tensor_mul(v2[:], r_im, kim_b)
                nc.vector.tensor_sub(p_re[:], v1[:], v2[:])
                nc.vector.tensor_mul(v3[:], r_re, kim_b)
                nc.vector.tensor_mul(v4[:], r_im, kre_b)
                nc.vector.tensor_add(p_im[:], v3[:], v4[:])

                # I1: C = P @ F+   [k1, n2]  (data-stationary)
                csrc = wp.tile([128, G, 256], dt_tt, tag="csb")
                for g in range(G // 2):
                    pc = pcp.tile([128, 2, 256], f32, tag="pc")
                    for i in range(2):
                        j = 2 * g + i
                        nc.tensor.matmul(
                            pc[:, i, :], p_re[:, j, :], fcmov[:, 128:384],
                            start=True, stop=False,
                        )
                        nc.tensor.matmul(
                            pc[:, i, :], p_im[:, j, :], fcmov[:, 0:256],
                            start=False, stop=True,
                        )
                    nc.scalar.copy(
                        out=csrc[:, 2 * g : 2 * g + 2, :], in_=pc[:]
                    )

                # T2 twiddle: C' = C * conj(T1)
                c_re = csrc[:, :, 0:128]
                c_im = csrc[:, :, 128:256]
                w1 = wp.tile([128, G, 128], dt_tt, tag="u1")
                w2 = wp.tile([128, G, 128], dt_tt, tag="u2")
                w3 = wp.tile([128, G, 128], dt_tt, tag="u3")
                w4 = wp.tile([128, G, 128], dt_tt, tag="u4")
                cp_re = wp.tile([128, G, 128], dt_tt, tag="cpre")
                cp_im = wp.tile([128, G, 128], dt_tt, tag="cpim")
                nc.vector.tensor_mul(w1[:], c_re, t1re_b)
                nc.vector.tensor_mul(w2[:], c_im, t1im_b)
                nc.vector.tensor_add(cp_re[:], w1[:], w2[:])
                nc.vector.tensor_mul(w3[:], c_re, t1im_b)
                nc.vector.tensor_mul(w4[:], c_im, t1re_b)
                nc.vector.tensor_sub(cp_im[:], w4[:], w3[:])

                # I2: y = Re(F+ @ C'), first 64 rows; 1/N folded into K
                ysb = op.tile([64, G, 128], f32, tag="ysb")
                for g in range(G // 2):
                    sl = slice(2 * g, 2 * g + 2)
                    py = pyp.tile([64, 2, 128], f32, tag="py")
                    nc.tensor.matmul(
                        py[:], gre[:], cp_re[:, sl, :], start=True, stop=False
                    )
                    nc.tensor.matmul(
                        py[:], gimn[:], cp_im[:, sl, :], start=False, stop=True
                    )
                    nc.scalar.copy(out=ysb[:, sl, :], in_=py[:])
                nc.sync.dma_start(
                    yw[c].rearrange("b (a n) -> a b n", n=128), ysb[:]
                )

    nc.compile()
    return nc


def _host_arrays():
    cst = _consts()
    F_cos, F_sin = cst["F_cos"], cst["F_sin"]
    Tw_cos, Tw_sin = cst["Tw_cos"], cst["Tw_sin"]

    np_tt = {"f32": np.float32, "f16": np.float16, "bf16": None}[TT_DT]
    if np_tt is None:
        import ml_dtypes

        np_tt = ml_dtypes.bfloat16
    np_mm = np.float32

    arrs = {}
    arrs["f1mov"] = np.concatenate([F_cos, -F_sin], axis=1).astype(np_mm)
    arrs["f2re"] = F_cos.astype(np_tt)
    arrs["f2im"] = (-F_sin).astype(np_tt)
    arrs["f2sin"] = F_sin.astype(np_tt)
    # fcmov = [F+_im_neg | F+_re | F+_im] = [-sin | cos | sin]
    arrs["fcmov"] = np.concatenate([-F_sin, F_cos, F_sin], axis=1).astype(np_tt)
    # 1/NFFT normalization lives in the host-side filter spectrum (keeps
    # every on-chip intermediate within fp16 range)
    arrs["gre"] = F_cos[:, :64].astype(np_tt)
    arrs["gimn"] = (-F_sin[:, :64]).astype(np_tt)
    arrs["t1re2"] = Tw_cos.astype(np_tt)
    arrs["t1im2"] = (-Tw_sin).astype(np_tt)
    return arrs, np_tt


def kernel(x: np.ndarray, filt: np.ndarray) -> np.ndarray:
    from concourse.bass_utils import run_bass_kernel_spmd

    assert x.shape == (B, L, D) and filt.shape == (D, L)
    x = np.ascontiguousarray(x, dtype=np.float32)
    filt = np.ascontiguousarray(filt, dtype=np.float32)

    consts, np_tt = _host_arrays()

    # filter spectrum: FFT of zero-padded filter; reshape(128,128) IS the
    # scrambled [k2,k1] layout produced by the on-device four-step forward.
    kpad = np.zeros((D, NFFT), np.float64)
    kpad[:, :L] = filt
    Kf = (np.fft.fft(kpad, axis=1) / NFFT).reshape(D, 128, 128)

    in_maps = []
    for ci in range(NC):
        sl = slice(ci * CPC, (ci + 1) * CPC)
        m = dict(consts)
        m["xw"] = np.ascontiguousarray(x[:, :, sl].transpose(2, 0, 1))
        m["kfre"] = np.ascontiguousarray(Kf[sl].real.astype(np_tt))
        m["kfim"] = np.ascontiguousarray(Kf[sl].imag.astype(np_tt))
        in_maps.append(m)

    nc = _build_program()
    res = run_bass_kernel_spmd(nc, in_maps, core_ids=list(range(NC)))

    y = np.empty((B, L, D), np.float32)
    for ci in range(NC):
        sl = slice(ci * CPC, (ci + 1) * CPC)
        y[:, :, sl] = res.results[ci]["yw"].transpose(1, 2, 0)
    return y


def run_profiled(inputs):
    """Build + run with NTFF tracing; returns BassKernelResults (test-only)."""
    from concourse.bass_utils import run_bass_kernel_spmd

    x = np.ascontiguousarray(inputs["x"], dtype=np.float32)
    filt = np.ascontiguousarray(inputs["filt"], dtype=np.float32)
    consts, np_tt = _host_arrays()
    kpad = np.zeros((D, NFFT), np.float64)
    kpad[:, :L] = filt
    Kf = (np.fft.fft(kpad, axis=1) / NFFT).reshape(D, 128, 128)
    in_maps = []
    for ci in range(NC):
        sl = slice(ci * CPC, (ci + 1) * CPC)
        m = dict(consts)
        m["xw"] = np.ascontiguousarray(x[:, :, sl].transpose(2, 0, 1))
        m["kfre"] = np.ascontiguousarray(Kf[sl].real.astype(np_tt))
        m["kfim"] = np.ascontiguousarray(Kf[sl].imag.astype(np_tt))
        in_maps.append(m)
    nc = _build_program()
    return run_bass_kernel_spmd(
        nc, in_maps, core_ids=list(range(NC)), trace=True
    )


if __name__ == "__main__":
    rng = np.random.default_rng(0)
    x = rng.standard_normal((B, L, D)).astype(np.float32)
    filt = rng.standard_normal((D, L)).astype(np.float32)
    y = kernel(x, filt)
    print("y", y.shape, y.dtype, float(np.abs(y).max()))
